# revision 16
# baseline (speedup 1.0000x reference)
"""Trainium2 kernel for nn_EnhancedHybridModel (hybrid MLP + 8-qubit circuit).

Reformulation (exact, up to f32 rounding):
  * BatchNorms are inference-mode -> folded into the adjacent Linear.
  * The quantum circuit after RY-encoding uses shared weights, so it is one
    fixed complex matrix M (256x256).  The encoded state is a REAL product
    vector s[b] = kron_i [cos(pre_i/2), -sin(pre_i/2)].
  * q_out = |M s|^2 @ Z  ->  y = [Re M; Im M] @ s  (512x256 matmul),
    then q_out @ W4eff.T folds with the Z-projection into M4 (512x32):
    h4 = relu(y^2 @ M4 + b4eff).

Data parallel over 8 NeuronCores: batch 65536 -> 8192 rows/core.

v2 layout strategy (vs the PE-transpose baseline):
  * pre/cos/sin are computed BATCH-major ([128 batch, small free]) via a
    data-stationary W3 matmul, so the tanh/sin scalar activations and the
    kron build all run on tiny free dims (engine time ~ free size).
  * cos/-sin come from the scalar engine's Sin table (one act table serves
    sin/tanh/relu/square), replacing the DVE polynomial chain.
  * The [batch, state] -> [state, batch] flips use the DMA XBAR transpose
    (16x128 tiles, 2-byte dtype) on the otherwise-idle DMA engines, freeing
    the PE of all transposes and the DVE of all PSUM->SBUF copies.
  * The final 16->1 matmul result is DMA'd PSUM->DRAM directly; b6 is added
    on the host.
  * PE instruction stream is interleaved so small matmuls hide their
    LDWEIGHTS under the 512-col quantum matmuls, keeping the PE dense (the
    PE only reaches its 2.4 GHz p-state when continuously busy).
"""

import numpy as np

import concourse.bass as bass
import concourse.mybir as mybir
import concourse.tile as tile
from concourse import bacc
from concourse.masks import make_identity
from concourse.bass_utils import run_bass_kernel_spmd

F32 = mybir.dt.float32
F16 = mybir.dt.float16
AF = mybir.ActivationFunctionType
ALU = mybir.AluOpType

N_CORES = 8
BATCH = 65536
B_CORE = BATCH // N_CORES  # 8192
COLS = 512  # batch columns per tile (one PSUM bank of f32)
NTILES = B_CORE // COLS  # 16

N_QUBITS = 8
N_LAYERS = 3
DIM = 256
EPS = 1e-5

# ---------------------------------------------------------------- host math

_idx = np.arange(DIM)
_CNOT_PERMS = []
for _i in range(N_QUBITS):
    for _j in range(_i + 1, N_QUBITS):
        _c = (_idx >> (N_QUBITS - 1 - _i)) & 1
        _CNOT_PERMS.append(np.where(_c == 1, _idx ^ (1 << (N_QUBITS - 1 - _j)), _idx))
_Z_SIGNS = np.stack(
    [1.0 - 2.0 * ((_idx >> (N_QUBITS - 1 - i)) & 1) for i in range(N_QUBITS)], axis=1
).astype(np.float64)


def _rx(t):
    c, s = np.cos(t / 2), -1j * np.sin(t / 2)
    return np.array([[c, s], [s, c]], np.complex128)


def _ry(t):
    c, s = np.cos(t / 2), np.sin(t / 2)
    return np.array([[c, -s], [s, c]], np.complex128)


def _rz(t):
    e = np.exp(-0.5j * t)
    return np.array([[e, 0], [0, np.conj(e)]], np.complex128)


def _apply_gate(M, G, w):
    # reference einsum('st,bpsq->bptq', U, state): state'[t] = sum_s U[s,t] state[s]
    left = 2**w
    Mr = M.reshape(left, 2, -1, DIM)
    return np.einsum("st,psqj->ptqj", G, Mr).reshape(DIM, DIM)


def _build_circuit_matrix(q_weights):
    qw = np.asarray(q_weights, np.float64)
    M = np.eye(DIM, dtype=np.complex128)
    for l in range(N_LAYERS):
        for i in range(N_QUBITS):
            M = _apply_gate(M, _rx(qw[l, i, 0]), i)
            M = _apply_gate(M, _ry(qw[l, i, 1]), i)
            M = _apply_gate(M, _rz(qw[l, i, 2]), i)
        for perm in _CNOT_PERMS:
            M = M[perm, :]
    return M


def _fold_bn(W, b, g, bt, m, v):
    sc = np.asarray(g, np.float64) / np.sqrt(np.asarray(v, np.float64) + EPS)
    Weff = sc[:, None] * np.asarray(W, np.float64)
    beff = (np.asarray(b, np.float64) - np.asarray(m, np.float64)) * sc + np.asarray(
        bt, np.float64
    )
    return Weff, beff


WCOLS = 1369


def _prep_consts(inputs):
    f = {k: np.asarray(v, np.float64) for k, v in inputs.items() if k != "x"}
    W1e, b1e = _fold_bn(f["W1"], f["b1"], f["g1"], f["bt1"], f["m1"], f["v1"])
    W2e, b2e = _fold_bn(f["W2"], f["b2"], f["g2"], f["bt2"], f["m2"], f["v2"])
    W4e, b4e = _fold_bn(f["W4"], f["b4"], f["g4"], f["bt4"], f["m4"], f["v4"])
    M = _build_circuit_matrix(f["q_weights"])
    C = np.concatenate([M.real, M.imag], axis=0)  # (512, 256)
    Zst = np.concatenate([_Z_SIGNS, _Z_SIGNS], axis=0)  # (512, 8)
    M4 = Zst @ W4e.T  # (512, 32)

    bf = np.float16
    # WPACK fp16 [128, 1369]: ct | w2 | w1 | w3aug | w5 | w6 | m4
    wpk = np.zeros((128, WCOLS), bf)
    CT = np.ascontiguousarray(C.T).astype(bf)  # (256, 512)
    wpk[:, 0:512] = CT[0:128]
    wpk[:, 512:1024] = CT[128:256]
    wpk[0:128, 1024:1088] = np.ascontiguousarray(W2e.T).astype(bf)
    wpk[0:16, 1088:1216] = np.ascontiguousarray(W1e.T).astype(bf)
    wpk[0:64, 1216:1224] = np.ascontiguousarray(f["W3"].T).astype(bf)  # (64, 8)
    wpk[64:65, 1216:1224] = np.asarray(f["b3"]).reshape(1, 8).astype(bf)
    wpk[0:32, 1224:1240] = np.ascontiguousarray(f["W5"].T).astype(bf)
    wpk[0:16, 1240:1241] = np.ascontiguousarray(f["W6"].T).astype(bf)
    M4b = M4.astype(bf)  # (512, 32)
    for c in range(4):
        wpk[:, 1241 + 32 * c : 1241 + 32 * (c + 1)] = M4b[128 * c : 128 * (c + 1)]
    # BIASES f32 [128, 6]: b1 b2 b4 b5 | pi/2 0
    bs = np.zeros((128, 6), np.float32)
    bs[0:128, 0] = b1e
    bs[0:64, 1] = b2e
    bs[0:32, 2] = b4e
    bs[0:16, 3] = f["b5"]
    bs[0:128, 4] = np.pi / 2
    return {"WPACK": wpk, "BIASES": bs}, float(np.asarray(f["b6"]).reshape(()))


# ------------------------------------------------------------- bass program


def _ap(t, offset, dims):
    """Custom free-dim access pattern on a tile: keep its partition dim."""
    a = t[:]
    return bass.AP(a.tensor, a.offset + offset, [list(a.ap[0])] + [list(d) for d in dims])


def _build_nc():
    nc = bacc.Bacc("TRN2", target_bir_lowering=False, debug=False)

    xt = nc.dram_tensor("xt", [16, B_CORE], F16, kind="ExternalInput")
    wpk_d = nc.dram_tensor("WPACK", [128, WCOLS], F16, kind="ExternalInput")
    bs_d = nc.dram_tensor("BIASES", [128, 6], F32, kind="ExternalInput")
    out_d = nc.dram_tensor("out", [1, B_CORE], F32, kind="ExternalOutput")

    HALF_PI = float(np.pi / 2)

    with tile.TileContext(nc) as tc:
        with (
            tc.tile_pool(name="const", bufs=1) as cp,
            tc.tile_pool(name="work", bufs=2) as wp,
            tc.tile_pool(name="pmlp", bufs=2, space="PSUM") as pmlp,
            tc.tile_pool(name="py", bufs=3, space="PSUM") as py,
            tc.tile_pool(name="pmlb", bufs=2, space="PSUM") as pmlb,
            tc.tile_pool(name="ptr", bufs=1, space="PSUM") as ptr,
        ):
            ident = cp.tile([128, 128], F16)
            make_identity(nc, ident[:])
            wpk = cp.tile([128, WCOLS], F16)
            # small weights (w1/w2/w3/w5/w6/m4) first so the MLP phases can
            # start while the big circuit matrix streams in behind them
            nc.scalar.dma_start(wpk[:, 1024:WCOLS], wpk_d[:, 1024:WCOLS])
            nc.scalar.dma_start(wpk[:, 0:1024], wpk_d[:, 0:1024])
            bs = cp.tile([128, 6], F32)
            tblpin = cp.tile([1, 1], F16, name="tblpin")
            nc.scalar.activation(tblpin[:], bs[0:1, 0:1], AF.Silu)
            ct = wpk[:, 0:1024]
            w2 = wpk[:, 1024:1088]
            w1 = wpk[0:16, 1088:1216]
            w3a = wpk[0:65, 1216:1224]
            w5 = wpk[0:32, 1224:1240]
            w6 = wpk[0:16, 1240:1241]
            m4 = wpk[:, 1241:1369]
            b1 = bs[0:128, 0:1]
            b2 = bs[0:64, 1:2]
            b4 = bs[0:32, 2:3]
            b5 = bs[0:16, 3:4]
            pi2 = bs[0:128, 4:5]
            zero = bs[0:128, 5:6]
            xg = []
            for g in range(4):
                xg.append(cp.tile([16, 4 * COLS], F16, name=f"xg{g}", tag=f"xg{g}"))
                nc.sync.dma_start(xg[g][:], xt[:, 4 * COLS * g : 4 * COLS * (g + 1)])
                if g == 0:
                    nc.sync.dma_start(bs[:], bs_d[:])

            out_all = cp.tile([1, B_CORE], F32)
            mm = nc.tensor.matmul

            # prewrite the ones-row into both h2 ring buffers (row 64 is the
            # bias row for the data-stationary W3 matmul; the per-tile act
            # only writes rows 0:64, so the ones persist across slot reuse)
            for _ in range(2):
                h2pre = wp.tile([65, COLS], F16, tag="h2", name="h2pre")
                nc.gpsimd.memset(h2pre[64:65, :], 1.0)

            h1 = [None] * NTILES
            h2 = [None] * NTILES
            pre = [None] * NTILES
            csA = [None] * NTILES
            sB = [None] * NTILES
            sT = [None] * NTILES
            sqa = [None] * NTILES
            sqb = [None] * NTILES
            h4 = [None] * NTILES
            h5 = [None] * NTILES
            yps = [None] * NTILES

            # compressed-prologue schedule: early phases run 2 tiles/iter
            # (PE is H-idle during pipe fill), steady state 1 tile/iter.
            PROLOG = {
                "A": {0: [0, 1], 1: [2, 3], 2: [4, 5], 3: [6]},
                "B": {0: [0], 1: [1, 2], 2: [3, 4], 3: [5]},
                "C": {1: [0, 1], 2: [2, 3], 3: [4]},
                "S": {1: [0, 1], 2: [2, 3], 3: [4]},
                "F": {1: [0], 2: [1, 2], 3: [3]},
                "G": {2: [0, 1], 3: [2]},
                "H": {3: [0]},
            }
            SL = dict(A=-3, B=-2, C=-1, S=-1, F=0, G=1, H=3, I=5, J=6, K=7)
            NITER = NTILES + SL["K"] + 1

            def sched(ph, t):
                if ph in PROLOG and t <= 3:
                    return [i for i in PROLOG[ph].get(t, []) if i < NTILES]
                if ph in PROLOG and t == 3:
                    return []
                i = t - SL[ph]
                if ph in PROLOG and t < 4:
                    return []
                return [i] if 0 <= i < NTILES else []

            def one(ph, t):
                lst = sched(ph, t)
                assert len(lst) <= 1
                return lst[0] if lst else None

            def sq_dst(i, c):
                return (sqa if c < 2 else sqb)[i][:, 512 * (c % 2) : 512 * (c % 2 + 1)]

            gmap = {}

            def doA(i):
                h1p = pmlp.tile([128, COLS], F32, tag="mlp", name="h1p")
                mm(h1p[:], w1, xg[i // 4][:, COLS * (i % 4) : COLS * (i % 4 + 1)])
                h1[i] = wp.tile([128, COLS], F16, tag="h1", name="h1")
                nc.vector.tensor_scalar(h1[i][:], h1p[:], b1, 0.0, ALU.add, ALU.max)

            def doB(i):
                h2p = pmlp.tile([64, COLS], F32, tag="mlp", name="h2p")
                mm(h2p[:], w2, h1[i][:])
                h2[i] = wp.tile([65, COLS], F16, tag="h2", name="h2")
                nc.vector.tensor_scalar(h2[i][0:64, :], h2p[:], b2, 0.0,
                                        ALU.add, ALU.max)

            def doC(i):
                prp = pmlp.tile([128, 32], F32, tag="mlp", name="prp")
                for c in range(4):
                    mm(prp[:, 8 * c : 8 * (c + 1)],
                       h2[i][0:65, 128 * c : 128 * (c + 1)], w3a)
                pre[i] = wp.tile([128, 32], F16, tag="pre", name="pre")
                nc.scalar.activation(pre[i][:], prp[:], AF.Tanh)

            def doS(i):
                csA[i] = wp.tile([128, 64], F16, tag="csA", name="csA")
                srcp = _ap(pre[i], 0, [[8, 4], [1, 8]])
                nc.scalar.activation(
                    _ap(csA[i], 0, [[16, 4], [1, 8]]), srcp, AF.Sin,
                    bias=pi2, scale=0.5)
                nc.scalar.activation(
                    _ap(csA[i], 8, [[16, 4], [1, 8]]), srcp, AF.Sin,
                    bias=zero, scale=-0.5)

            def doF(i):
                qp = wp.tile([128, 64], F16, tag="qp", name="qp")
                for a in range(2):
                    nc.vector.tensor_mul(
                        _ap(qp, 2 * a, [[16, 4], [4, 4], [1, 2]]),
                        _ap(csA[i], 8 * a, [[16, 4], [2, 4], [0, 2]]),
                        _ap(csA[i], 1, [[16, 4], [2, 4], [8, 2]]),
                    )
                uv = wp.tile([128, 128], F16, tag="uv", name="uv")
                nc.gpsimd.tensor_mul(
                    _ap(uv, 0, [[16, 8], [4, 4], [1, 4]]),
                    _ap(qp, 0, [[8, 8], [1, 4], [0, 4]]),
                    _ap(qp, 4, [[8, 8], [0, 4], [1, 4]]),
                )
                sB[i] = wp.tile([128, 1024], F16, tag="sB", name="sB", bufs=3)
                nc.gpsimd.tensor_mul(
                    _ap(sB[i], 0, [[256, 2], [16, 16], [1, 16]]),
                    _ap(uv, 0, [[32, 2], [1, 16], [0, 16]]),
                    _ap(uv, 16, [[32, 2], [0, 16], [1, 16]]),
                )
                nc.gpsimd.tensor_mul(
                    _ap(sB[i], 512, [[256, 2], [16, 16], [1, 16]]),
                    _ap(uv, 64, [[32, 2], [1, 16], [0, 16]]),
                    _ap(uv, 80, [[32, 2], [0, 16], [1, 16]]),
                )

            # gp layout: [b2h0, b3h0, b2h1, b3h1, b1h1]
            _GQ = {(2, 0): 0, (3, 0): 1, (2, 1): 2, (3, 1): 3, (1, 1): 4}

            def doGsync(i):
                sT[i] = wp.tile([128, 1024], F16, tag="sT", name="sT", bufs=3)
                nc.sync.dma_start_transpose(sT[i][:, 0:128], sB[i][:, 0:128])
                nc.sync.dma_start_transpose(sT[i][:, 512:640], sB[i][:, 128:256])
                nc.sync.dma_start_transpose(sT[i][:, 128:256], sB[i][:, 256:384])
                gmap[i] = ptr.tile([128, 640], F16, tag="tr", name="gp")

            def doGtrans(i, b):
                for h in range(2):
                    if (b, h) not in _GQ:
                        continue
                    q = _GQ[(b, h)]
                    nc.tensor.transpose(
                        gmap[i][:, 128 * q : 128 * (q + 1)],
                        sB[i][:, 256 * b + 128 * h : 256 * b + 128 * h + 128],
                        ident[:])

            def doGcopy(i):
                nc.vector.tensor_copy(
                    _ap(sT[i], 256, [[512, 2], [1, 256]]),
                    _ap(gmap[i], 0, [[256, 2], [1, 256]]))
                nc.vector.tensor_copy(sT[i][:, 640:768], gmap[i][:, 512:640])

            def doHalloc(i):
                yps[i] = []
                sqa[i] = wp.tile([128, 1024], F16, tag="sqa", name="sqa", bufs=3)
                sqb[i] = wp.tile([128, 1024], F16, tag="sqb", name="sqb", bufs=3)

            def hpair(i, c):
                yp = py.tile([128, COLS], F32, tag="y", name="yp")
                mm(yp[:], ct[:, 128 * c : 128 * (c + 1)], sT[i][:, 0:512],
                   start=True, stop=False)
                mm(yp[:], ct[:, 512 + 128 * c : 512 + 128 * (c + 1)],
                   sT[i][:, 512:1024], start=False, stop=True)
                yps[i].append(yp)

            def hsq(i, c):
                if c == 1:
                    yc = wp.tile([128, COLS], F16, tag="yc", name="yc", bufs=2)
                    nc.vector.tensor_copy(yc[:], yps[i][c][:])
                    nc.vector.tensor_mul(sq_dst(i, c), yc[:], yc[:])
                else:
                    nc.scalar.activation(sq_dst(i, c), yps[i][c][:], AF.Square)

            def doI(i):
                h4p = pmlb.tile([32, COLS], F32, tag="mlb", name="h4p")
                for c in range(4):
                    srcq = (sqa if c < 2 else sqb)[i][:, 512 * (c % 2) : 512 * (c % 2 + 1)]
                    mm(h4p[:], m4[:, 32 * c : 32 * (c + 1)], srcq,
                       start=(c == 0), stop=(c == 3))
                h4[i] = wp.tile([32, COLS], F16, tag="h4", name="h4")
                nc.scalar.activation(h4[i][:], h4p[:], AF.Relu, bias=b4)

            def doJ(i):
                h5p = pmlb.tile([16, COLS], F32, tag="mlb", name="h5p")
                mm(h5p[:], w5, h4[i][:])
                h5[i] = wp.tile([16, COLS], F16, tag="h5", name="h5")
                nc.scalar.activation(h5[i][:], h5p[:], AF.Relu, bias=b5)

            def doK(i):
                kp = pmlb.tile([1, COLS], F32, tag="mlb", name="kp")
                mm(kp[:], w6, h5[i][:])
                nc.vector.tensor_copy(out_all[0:1, COLS * i : COLS * (i + 1)], kp[:])
                nc.sync.dma_start(out_d[:, COLS * i : COLS * (i + 1)],
                                  out_all[0:1, COLS * i : COLS * (i + 1)])

            for t in range(NITER):
                if t <= 3:
                    # prologue: sequential, PE has slack
                    for i in sched("A", t):
                        doA(i)
                    for i in sched("B", t):
                        doB(i)
                    for i in sched("C", t):
                        doC(i)
                    for i in sched("S", t):
                        doS(i)
                    for i in sched("F", t):
                        doF(i)
                    for i in sched("G", t):
                        doGsync(i)
                        for b in (1, 2, 3):
                            doGtrans(i, b)
                        doGcopy(i)
                    for i in sched("H", t):
                        doHalloc(i)
                        for c in range(4):
                            hpair(i, c)
                            hsq(i, c)
                    continue

                iA = one("A", t); iB = one("B", t); iC = one("C", t)
                iS = one("S", t); iF = one("F", t); iG = one("G", t)
                iH = one("H", t); iI = one("I", t); iJ = one("J", t)
                iK = one("K", t)

                if iG is not None:
                    doGsync(iG)
                if iH is not None:
                    doHalloc(iH)

                if iG is not None:
                    doGtrans(iG, 1)
                if iH is not None:
                    hpair(iH, 0)
                    hsq(iH, 0)
                if iA is not None:
                    doA(iA)
                if iH is not None:
                    hpair(iH, 1)
                    hsq(iH, 1)
                if iG is not None:
                    doGtrans(iG, 2)
                if iB is not None:
                    doB(iB)
                if iH is not None:
                    hpair(iH, 2)
                    hsq(iH, 2)
                if iG is not None:
                    doGtrans(iG, 3)
                    doGcopy(iG)
                if iC is not None:
                    doC(iC)
                if iH is not None:
                    hpair(iH, 3)
                    hsq(iH, 3)
                if iS is not None:
                    doS(iS)
                if iF is not None:
                    doF(iF)
                if iI is not None:
                    doI(iI)
                if iJ is not None:
                    doJ(iJ)
                if iK is not None:
                    doK(iK)

    nc.compile()
    return nc


_NC_CACHE = []

# test-harness hooks (unused in grading): set _TRACE to profile; the full
# BassKernelResults of the last run lands in _LAST_RESULTS[0].
_TRACE = False
_LAST_RESULTS = []


def _get_nc():
    if not _NC_CACHE:
        _NC_CACHE.append(_build_nc())
    return _NC_CACHE[0]


def kernel(**inputs):
    consts, b6 = _prep_consts(inputs)
    x = np.asarray(inputs["x"], np.float32)  # (65536, 16)
    xt_full = np.ascontiguousarray(x.T.astype(np.float16))  # (16, 65536)

    nc = _get_nc()
    in_maps = []
    for c in range(N_CORES):
        m = {"xt": np.ascontiguousarray(xt_full[:, c * B_CORE : (c + 1) * B_CORE])}
        m.update(consts)
        in_maps.append(m)
    res = run_bass_kernel_spmd(nc, in_maps, list(range(N_CORES)), trace=_TRACE)
    _LAST_RESULTS.clear()
    _LAST_RESULTS.append(res)
    out = np.concatenate([r["out"].reshape(B_CORE) for r in res.results])
    return (out.reshape(BATCH, 1) + b6).astype(np.float32)


# revision 17
# speedup vs baseline: 1.1747x; 1.1747x over previous
"""Trainium2 kernel for nn_EnhancedHybridModel (hybrid MLP + 8-qubit circuit).

Reformulation (exact, up to f32 rounding):
  * BatchNorms are inference-mode -> folded into the adjacent Linear.
  * The quantum circuit after RY-encoding uses shared weights, so it is one
    fixed complex matrix M (256x256).  The encoded state is a REAL product
    vector s[b] = kron_i [cos(pre_i/2), -sin(pre_i/2)].
  * q_out = |M s|^2 @ Z  ->  y = [Re M; Im M] @ s  (512x256 matmul),
    then q_out @ W4eff.T folds with the Z-projection into M4 (512x32):
    h4 = relu(y^2 @ M4 + b4eff).

Data parallel over 8 NeuronCores: batch 65536 -> 8192 rows/core.

v2 layout strategy (vs the PE-transpose baseline):
  * pre/cos/sin are computed BATCH-major ([128 batch, small free]) via a
    data-stationary W3 matmul, so the tanh/sin scalar activations and the
    kron build all run on tiny free dims (engine time ~ free size).
  * cos/-sin come from the scalar engine's Sin table (one act table serves
    sin/tanh/relu/square), replacing the DVE polynomial chain.
  * The [batch, state] -> [state, batch] flips use the DMA XBAR transpose
    (16x128 tiles, 2-byte dtype) on the otherwise-idle DMA engines, freeing
    the PE of all transposes and the DVE of all PSUM->SBUF copies.
  * The final 16->1 matmul result is DMA'd PSUM->DRAM directly; b6 is added
    on the host.
  * PE instruction stream is interleaved so small matmuls hide their
    LDWEIGHTS under the 512-col quantum matmuls, keeping the PE dense (the
    PE only reaches its 2.4 GHz p-state when continuously busy).
"""

import numpy as np

import concourse.bass as bass
import concourse.mybir as mybir
import concourse.tile as tile
from concourse import bacc
from concourse.masks import make_identity
from concourse.bass_utils import run_bass_kernel_spmd

F32 = mybir.dt.float32
F16 = mybir.dt.float16
AF = mybir.ActivationFunctionType
ALU = mybir.AluOpType

N_CORES = 8
BATCH = 65536
B_CORE = BATCH // N_CORES  # 8192
COLS = 512  # batch columns per tile (one PSUM bank of f32)
NTILES = B_CORE // COLS  # 16

N_QUBITS = 8
N_LAYERS = 3
DIM = 256
EPS = 1e-5

# ---------------------------------------------------------------- host math

_idx = np.arange(DIM)
_CNOT_PERMS = []
for _i in range(N_QUBITS):
    for _j in range(_i + 1, N_QUBITS):
        _c = (_idx >> (N_QUBITS - 1 - _i)) & 1
        _CNOT_PERMS.append(np.where(_c == 1, _idx ^ (1 << (N_QUBITS - 1 - _j)), _idx))
_Z_SIGNS = np.stack(
    [1.0 - 2.0 * ((_idx >> (N_QUBITS - 1 - i)) & 1) for i in range(N_QUBITS)], axis=1
).astype(np.float64)


def _rx(t):
    c, s = np.cos(t / 2), -1j * np.sin(t / 2)
    return np.array([[c, s], [s, c]], np.complex128)


def _ry(t):
    c, s = np.cos(t / 2), np.sin(t / 2)
    return np.array([[c, -s], [s, c]], np.complex128)


def _rz(t):
    e = np.exp(-0.5j * t)
    return np.array([[e, 0], [0, np.conj(e)]], np.complex128)


def _apply_gate(M, G, w):
    # reference einsum('st,bpsq->bptq', U, state): state'[t] = sum_s U[s,t] state[s]
    left = 2**w
    Mr = M.reshape(left, 2, -1, DIM)
    return np.einsum("st,psqj->ptqj", G, Mr).reshape(DIM, DIM)


def _build_circuit_matrix(q_weights):
    qw = np.asarray(q_weights, np.float64)
    M = np.eye(DIM, dtype=np.complex128)
    for l in range(N_LAYERS):
        for i in range(N_QUBITS):
            M = _apply_gate(M, _rx(qw[l, i, 0]), i)
            M = _apply_gate(M, _ry(qw[l, i, 1]), i)
            M = _apply_gate(M, _rz(qw[l, i, 2]), i)
        for perm in _CNOT_PERMS:
            M = M[perm, :]
    return M


def _fold_bn(W, b, g, bt, m, v):
    sc = np.asarray(g, np.float64) / np.sqrt(np.asarray(v, np.float64) + EPS)
    Weff = sc[:, None] * np.asarray(W, np.float64)
    beff = (np.asarray(b, np.float64) - np.asarray(m, np.float64)) * sc + np.asarray(
        bt, np.float64
    )
    return Weff, beff


WCOLS = 1369


def _prep_consts(inputs):
    f = {k: np.asarray(v, np.float64) for k, v in inputs.items() if k != "x"}
    W1e, b1e = _fold_bn(f["W1"], f["b1"], f["g1"], f["bt1"], f["m1"], f["v1"])
    W2e, b2e = _fold_bn(f["W2"], f["b2"], f["g2"], f["bt2"], f["m2"], f["v2"])
    W4e, b4e = _fold_bn(f["W4"], f["b4"], f["g4"], f["bt4"], f["m4"], f["v4"])
    M = _build_circuit_matrix(f["q_weights"])
    C = np.concatenate([M.real, M.imag], axis=0)  # (512, 256)
    Zst = np.concatenate([_Z_SIGNS, _Z_SIGNS], axis=0)  # (512, 8)
    M4 = Zst @ W4e.T  # (512, 32)

    bf = np.float16
    # WPACK fp16 [128, 1369]: ct | w2 | w1 | w3aug | w5 | w6 | m4
    wpk = np.zeros((128, WCOLS), bf)
    CT = np.ascontiguousarray(C.T).astype(bf)  # (256, 512)
    wpk[:, 0:512] = CT[0:128]
    wpk[:, 512:1024] = CT[128:256]
    wpk[0:128, 1024:1088] = np.ascontiguousarray(W2e.T).astype(bf)
    wpk[0:16, 1088:1216] = np.ascontiguousarray(W1e.T).astype(bf)
    wpk[0:64, 1216:1224] = np.ascontiguousarray(f["W3"].T).astype(bf)  # (64, 8)
    wpk[64:65, 1216:1224] = np.asarray(f["b3"]).reshape(1, 8).astype(bf)
    wpk[0:32, 1224:1240] = np.ascontiguousarray(f["W5"].T).astype(bf)
    wpk[0:16, 1240:1241] = np.ascontiguousarray(f["W6"].T).astype(bf)
    M4b = M4.astype(bf)  # (512, 32)
    for c in range(4):
        wpk[:, 1241 + 32 * c : 1241 + 32 * (c + 1)] = M4b[128 * c : 128 * (c + 1)]
    # BIASES f32 [128, 6]: b1 b2 b4 b5 | pi/2 0
    bs = np.zeros((128, 6), np.float32)
    bs[0:128, 0] = b1e
    bs[0:64, 1] = b2e
    bs[0:32, 2] = b4e
    bs[0:16, 3] = f["b5"]
    bs[0:128, 4] = np.pi / 2
    return {"WPACK": wpk, "BIASES": bs}, float(np.asarray(f["b6"]).reshape(()))


# ------------------------------------------------------------- bass program


def _ap(t, offset, dims):
    """Custom free-dim access pattern on a tile: keep its partition dim."""
    a = t[:]
    return bass.AP(a.tensor, a.offset + offset, [list(a.ap[0])] + [list(d) for d in dims])


def _build_nc():
    nc = bacc.Bacc("TRN2", target_bir_lowering=False, debug=False)

    xt = nc.dram_tensor("xt", [16, B_CORE], F16, kind="ExternalInput")
    wpk_d = nc.dram_tensor("WPACK", [128, WCOLS], F16, kind="ExternalInput")
    bs_d = nc.dram_tensor("BIASES", [128, 6], F32, kind="ExternalInput")
    out_d = nc.dram_tensor("out", [1, B_CORE], F32, kind="ExternalOutput")

    HALF_PI = float(np.pi / 2)

    with tile.TileContext(nc) as tc:
        with (
            tc.tile_pool(name="const", bufs=1) as cp,
            tc.tile_pool(name="work", bufs=2) as wp,
            tc.tile_pool(name="pmlp", bufs=2, space="PSUM") as pmlp,
            tc.tile_pool(name="py", bufs=3, space="PSUM") as py,
            tc.tile_pool(name="pmlb", bufs=2, space="PSUM") as pmlb,
            tc.tile_pool(name="ptr", bufs=1, space="PSUM") as ptr,
        ):
            ident = cp.tile([128, 128], F16)
            make_identity(nc, ident[:])
            wpk = cp.tile([128, WCOLS], F16)
            # small weights (w1/w2/w3/w5/w6/m4) first so the MLP phases can
            # start while the big circuit matrix streams in behind them
            nc.scalar.dma_start(wpk[:, 1024:WCOLS], wpk_d[:, 1024:WCOLS])
            nc.scalar.dma_start(wpk[:, 0:1024], wpk_d[:, 0:1024])
            bs = cp.tile([128, 6], F32)
            tblpin = cp.tile([1, 1], F16, name="tblpin")
            nc.scalar.activation(tblpin[:], bs[0:1, 0:1], AF.Silu)
            ct = wpk[:, 0:1024]
            w2 = wpk[:, 1024:1088]
            w1 = wpk[0:16, 1088:1216]
            w3a = wpk[0:65, 1216:1224]
            w5 = wpk[0:32, 1224:1240]
            w6 = wpk[0:16, 1240:1241]
            m4 = wpk[:, 1241:1369]
            b1 = bs[0:128, 0:1]
            b2 = bs[0:64, 1:2]
            b4 = bs[0:32, 2:3]
            b5 = bs[0:16, 3:4]
            pi2 = bs[0:128, 4:5]
            zero = bs[0:128, 5:6]
            xg = []
            for g in range(4):
                xg.append(cp.tile([16, 4 * COLS], F16, name=f"xg{g}", tag=f"xg{g}"))
                nc.sync.dma_start(xg[g][:], xt[:, 4 * COLS * g : 4 * COLS * (g + 1)])
                if g == 0:
                    nc.sync.dma_start(bs[:], bs_d[:])

            out_all = cp.tile([1, B_CORE], F32)
            mm = nc.tensor.matmul

            # prewrite the ones-row into both h2 ring buffers (row 64 is the
            # bias row for the data-stationary W3 matmul; the per-tile act
            # only writes rows 0:64, so the ones persist across slot reuse)
            for _ in range(2):
                h2pre = wp.tile([65, COLS], F16, tag="h2", name="h2pre")
                nc.gpsimd.memset(h2pre[64:65, :], 1.0)

            h1 = [None] * NTILES
            h2 = [None] * NTILES
            pre = [None] * NTILES
            csA = [None] * NTILES
            sB = [None] * NTILES
            sT = [None] * NTILES
            sqa = [None] * NTILES
            sqb = [None] * NTILES
            h4 = [None] * NTILES
            h5 = [None] * NTILES
            yps = [None] * NTILES

            # compressed-prologue schedule: early phases run 2 tiles/iter
            # (PE is H-idle during pipe fill), steady state 1 tile/iter.
            PROLOG = {
                "A": {0: [0, 1], 1: [2, 3], 2: [4, 5], 3: [6]},
                "B": {0: [0], 1: [1, 2], 2: [3, 4], 3: [5]},
                "C": {1: [0, 1], 2: [2, 3], 3: [4]},
                "S": {1: [0, 1], 2: [2, 3], 3: [4]},
                "F": {1: [0], 2: [1, 2], 3: [3]},
                "G": {2: [0, 1], 3: [2]},
                "H": {3: [0]},
            }
            SL = dict(A=-3, B=-2, C=-1, S=-1, F=0, G=1, H=3, I=5, J=6, K=7)
            NITER = NTILES + SL["K"] + 1

            def sched(ph, t):
                if ph in PROLOG and t <= 3:
                    return [i for i in PROLOG[ph].get(t, []) if i < NTILES]
                if ph in PROLOG and t == 3:
                    return []
                i = t - SL[ph]
                if ph in PROLOG and t < 4:
                    return []
                return [i] if 0 <= i < NTILES else []

            def one(ph, t):
                lst = sched(ph, t)
                assert len(lst) <= 1
                return lst[0] if lst else None

            def sq_dst(i, c):
                return (sqa if c < 2 else sqb)[i][:, 512 * (c % 2) : 512 * (c % 2 + 1)]

            gmap = {}

            def doA(i):
                h1p = pmlp.tile([128, COLS], F32, tag="mlp", name="h1p")
                mm(h1p[:], w1, xg[i // 4][:, COLS * (i % 4) : COLS * (i % 4 + 1)])
                h1[i] = wp.tile([128, COLS], F16, tag="h1", name="h1")
                nc.vector.tensor_scalar(h1[i][:], h1p[:], b1, 0.0, ALU.add, ALU.max)

            def doB(i):
                h2p = pmlp.tile([64, COLS], F32, tag="mlp", name="h2p")
                mm(h2p[:], w2, h1[i][:])
                h2[i] = wp.tile([65, COLS], F16, tag="h2", name="h2")
                nc.vector.tensor_scalar(h2[i][0:64, :], h2p[:], b2, 0.0,
                                        ALU.add, ALU.max)

            def doC(i):
                prp = pmlp.tile([128, 32], F32, tag="mlp", name="prp")
                for c in range(4):
                    mm(prp[:, 8 * c : 8 * (c + 1)],
                       h2[i][0:65, 128 * c : 128 * (c + 1)], w3a)
                pre[i] = wp.tile([128, 32], F16, tag="pre", name="pre")
                nc.scalar.activation(pre[i][:], prp[:], AF.Tanh)

            def doS(i):
                csA[i] = wp.tile([128, 64], F16, tag="csA", name="csA")
                srcp = _ap(pre[i], 0, [[8, 4], [1, 8]])
                nc.scalar.activation(
                    _ap(csA[i], 0, [[16, 4], [1, 8]]), srcp, AF.Sin,
                    bias=pi2, scale=0.5)
                nc.scalar.activation(
                    _ap(csA[i], 8, [[16, 4], [1, 8]]), srcp, AF.Sin,
                    bias=zero, scale=-0.5)

            def doF(i):
                qp = wp.tile([128, 64], F16, tag="qp", name="qp")
                for a in range(2):
                    nc.vector.tensor_mul(
                        _ap(qp, 2 * a, [[16, 4], [4, 4], [1, 2]]),
                        _ap(csA[i], 8 * a, [[16, 4], [2, 4], [0, 2]]),
                        _ap(csA[i], 1, [[16, 4], [2, 4], [8, 2]]),
                    )
                uv = wp.tile([128, 128], F16, tag="uv", name="uv")
                nc.gpsimd.tensor_mul(
                    _ap(uv, 0, [[16, 8], [4, 4], [1, 4]]),
                    _ap(qp, 0, [[8, 8], [1, 4], [0, 4]]),
                    _ap(qp, 4, [[8, 8], [0, 4], [1, 4]]),
                )
                sB[i] = wp.tile([128, 1024], F16, tag="sB", name="sB", bufs=3)
                nc.gpsimd.tensor_mul(
                    _ap(sB[i], 0, [[256, 2], [16, 16], [1, 16]]),
                    _ap(uv, 0, [[32, 2], [1, 16], [0, 16]]),
                    _ap(uv, 16, [[32, 2], [0, 16], [1, 16]]),
                )
                nc.gpsimd.tensor_mul(
                    _ap(sB[i], 512, [[256, 2], [16, 16], [1, 16]]),
                    _ap(uv, 64, [[32, 2], [1, 16], [0, 16]]),
                    _ap(uv, 80, [[32, 2], [0, 16], [1, 16]]),
                )

            # gp layout: [b2h0, b3h0, b2h1, b3h1, b1h1]
            _GQ = {(2, 0): 0, (3, 0): 1, (2, 1): 2, (3, 1): 3, (1, 1): 4}

            def doGsync(i):
                sT[i] = wp.tile([128, 1024], F16, tag="sT", name="sT", bufs=3)
                nc.sync.dma_start_transpose(sT[i][:, 0:128], sB[i][:, 0:128])
                nc.sync.dma_start_transpose(sT[i][:, 512:640], sB[i][:, 128:256])
                nc.sync.dma_start_transpose(sT[i][:, 128:256], sB[i][:, 256:384])
                gmap[i] = ptr.tile([128, 640], F16, tag="tr", name="gp")

            def doGtrans(i, b):
                for h in range(2):
                    if (b, h) not in _GQ:
                        continue
                    q = _GQ[(b, h)]
                    nc.tensor.transpose(
                        gmap[i][:, 128 * q : 128 * (q + 1)],
                        sB[i][:, 256 * b + 128 * h : 256 * b + 128 * h + 128],
                        ident[:])

            def doGcopy(i):
                nc.vector.tensor_copy(
                    _ap(sT[i], 256, [[512, 2], [1, 256]]),
                    _ap(gmap[i], 0, [[256, 2], [1, 256]]))
                nc.vector.tensor_copy(sT[i][:, 640:768], gmap[i][:, 512:640])

            def doHalloc(i):
                yps[i] = []
                sqa[i] = wp.tile([128, 1024], F16, tag="sqa", name="sqa", bufs=3)
                sqb[i] = wp.tile([128, 1024], F16, tag="sqb", name="sqb", bufs=3)

            def hpair(i, c):
                yp = py.tile([128, COLS], F32, tag="y", name="yp")
                mm(yp[:], ct[:, 128 * c : 128 * (c + 1)], sT[i][:, 0:512],
                   start=True, stop=False)
                mm(yp[:], ct[:, 512 + 128 * c : 512 + 128 * (c + 1)],
                   sT[i][:, 512:1024], start=False, stop=True)
                yps[i].append(yp)

            def hsq(i, c):
                if c == 1:
                    yc = wp.tile([128, COLS], F16, tag="yc", name="yc", bufs=2)
                    nc.vector.tensor_copy(yc[:], yps[i][c][:])
                    nc.vector.tensor_mul(sq_dst(i, c), yc[:], yc[:])
                else:
                    nc.scalar.activation(sq_dst(i, c), yps[i][c][:], AF.Square)

            def doI(i):
                h4p = pmlb.tile([32, COLS], F32, tag="mlb", name="h4p")
                for c in range(4):
                    srcq = (sqa if c < 2 else sqb)[i][:, 512 * (c % 2) : 512 * (c % 2 + 1)]
                    mm(h4p[:], m4[:, 32 * c : 32 * (c + 1)], srcq,
                       start=(c == 0), stop=(c == 3))
                h4[i] = wp.tile([32, COLS], F16, tag="h4", name="h4")
                nc.scalar.activation(h4[i][:], h4p[:], AF.Relu, bias=b4)

            def doJ(i):
                h5p = pmlb.tile([16, COLS], F32, tag="mlb", name="h5p")
                mm(h5p[:], w5, h4[i][:])
                h5[i] = wp.tile([16, COLS], F16, tag="h5", name="h5")
                nc.scalar.activation(h5[i][:], h5p[:], AF.Relu, bias=b5)

            def doK(i):
                kp = pmlb.tile([1, COLS], F32, tag="mlb", name="kp")
                mm(kp[:], w6, h5[i][:])
                nc.vector.tensor_copy(out_all[0:1, COLS * i : COLS * (i + 1)], kp[:])
                if i >= NTILES - 4:
                    nc.scalar.dma_start(out_d[:, COLS * i : COLS * (i + 1)],
                                        out_all[0:1, COLS * i : COLS * (i + 1)])
                elif i == NTILES - 5:
                    nc.scalar.dma_start(out_d[:, 0 : COLS * (NTILES - 4)],
                                        out_all[0:1, 0 : COLS * (NTILES - 4)])

            for t in range(NITER):
                if t <= 3:
                    # prologue: sequential, PE has slack
                    for i in sched("A", t):
                        doA(i)
                    for i in sched("B", t):
                        doB(i)
                    for i in sched("C", t):
                        doC(i)
                    for i in sched("S", t):
                        doS(i)
                    for i in sched("F", t):
                        doF(i)
                    for i in sched("G", t):
                        doGsync(i)
                        for b in (1, 2, 3):
                            doGtrans(i, b)
                        doGcopy(i)
                    for i in sched("H", t):
                        doHalloc(i)
                        for c in range(4):
                            hpair(i, c)
                            hsq(i, c)
                    continue

                iA = one("A", t); iB = one("B", t); iC = one("C", t)
                iS = one("S", t); iF = one("F", t); iG = one("G", t)
                iH = one("H", t); iI = one("I", t); iJ = one("J", t)
                iK = one("K", t)

                if iG is not None:
                    doGsync(iG)
                if iH is not None:
                    doHalloc(iH)

                if iG is not None:
                    doGtrans(iG, 1)
                if iH is not None:
                    hpair(iH, 0)
                    hsq(iH, 0)
                if iA is not None:
                    doA(iA)
                if iH is not None:
                    hpair(iH, 1)
                    hsq(iH, 1)
                if iG is not None:
                    doGtrans(iG, 2)
                if iB is not None:
                    doB(iB)
                if iH is not None:
                    hpair(iH, 2)
                    hsq(iH, 2)
                if iG is not None:
                    doGtrans(iG, 3)
                    doGcopy(iG)
                if iC is not None:
                    doC(iC)
                if iH is not None:
                    hpair(iH, 3)
                    hsq(iH, 3)
                if iS is not None:
                    doS(iS)
                if iF is not None:
                    doF(iF)
                if iI is not None:
                    doI(iI)
                if iJ is not None:
                    doJ(iJ)
                if iK is not None:
                    doK(iK)

    nc.compile()
    return nc


_NC_CACHE = []

# test-harness hooks (unused in grading): set _TRACE to profile; the full
# BassKernelResults of the last run lands in _LAST_RESULTS[0].
_TRACE = False
_LAST_RESULTS = []


def _get_nc():
    if not _NC_CACHE:
        _NC_CACHE.append(_build_nc())
    return _NC_CACHE[0]


def kernel(**inputs):
    consts, b6 = _prep_consts(inputs)
    x = np.asarray(inputs["x"], np.float32)  # (65536, 16)
    xt_full = np.ascontiguousarray(x.T.astype(np.float16))  # (16, 65536)

    nc = _get_nc()
    in_maps = []
    for c in range(N_CORES):
        m = {"xt": np.ascontiguousarray(xt_full[:, c * B_CORE : (c + 1) * B_CORE])}
        m.update(consts)
        in_maps.append(m)
    res = run_bass_kernel_spmd(nc, in_maps, list(range(N_CORES)), trace=_TRACE)
    _LAST_RESULTS.clear()
    _LAST_RESULTS.append(res)
    out = np.concatenate([r["out"].reshape(B_CORE) for r in res.results])
    return (out.reshape(BATCH, 1) + b6).astype(np.float32)
